# revision 43
# baseline (speedup 1.0000x reference)
"""Causal self-attention (T=2048, C=1024, H=16) on 8 trn2 NeuronCores.

Tensor-parallel over heads: core i computes heads 2i, 2i+1 (q/k/v rows
128i:128i+128 of each 1024-row block of wqkv_w, proj_w columns
128i:128i+128), producing a partial output projection; partials are summed
on the host (the all-reduce of the sharding hint).

Per-core Bass/Tile kernel, bf16 matmuls with fp32 PSUM accumulation.
Layout puts head 0's attention pipeline on partitions 0-63 and head 1's on
64-127 so the two heads' K=64 matmuls land in disjoint PE row groups
(auto tile_position from base partitions) and run concurrently:
  B. warmup matmuls on the identity tile keep HAM's activity window busy
     while the x DMAs stream, so stage B starts at K=8/8 (2.4GHz).
     qkvT[j, t] = wqkv.T @ xT, contraction-tile outer so matmuls chase the
     x DMAs; q rows pre-scaled by 1/sqrt(C) on the host. v's 128x128 PE
     transposes produce both heads' v_aug tiles at once and are interleaved
     with the v matmuls so they never form a transpose-only PE window.
  D. per 512-col t-chunk: sT[k, 2, t] = kT.T @ qT (both heads, one wide
     2-bank PSUM tile) -> ONE wide exp per j on ScalarE (bf16 out, no
     max-subtraction: |scores| < ~1) -> causal affine_select on gpsimd
     (diagonal k-tiles only, both heads in one 3D op) -> PV:
       pvA[0:65]  += v_aug0.T @ w0   (M=65, ones col = head-0 denominator)
       den1[96:97]+= ones.T   @ w1   (M=1 packed into PE col group 3,
                                      concurrent with the pvA matmul)
       pvB[64:128]+= v_aug1.T @ w1   (M=64 at base partition 64)
     Normalize on DVE: denominators broadcast via two concurrent K=1
     matmuls (row groups 2/3), reciprocal_approx_fast, one multiply per
     head -> attn_pair[128, T] bf16 (h0 rows 0-63, h1 rows 64-127).
     Previous chunk's normalize/proj spread through the j-loop as PE
     filler while ScalarE (exp) is the per-step bottleneck.
  E. partialT[o, t] = projT.T @ attn_pair: per o-tile, two concurrent
     K=64 matmuls (row groups {2,3} then {0,1}; the rows-64-127 one is
     emitted first / start=True because its drain path is shorter).
"""

import sys

if "/opt/trn_rl_repo" not in sys.path:
    sys.path.insert(0, "/opt/trn_rl_repo")

import os

import ml_dtypes
import numpy as np

_DBG = set(os.environ.get("KDBG", "").split(","))


class _FakeWide:
    """Pair of 1-bank PSUM tiles indexed like a [128, 2, CH] wide tile."""

    def __init__(self, tiles):
        self.tiles = tiles

    def __getitem__(self, idx):
        assert len(idx) == 3
        return self.tiles[idx[1]][idx[0], idx[2]]

T = 2048
C = 1024
CH = 512  # t-chunk width (one PSUM bank of fp32)
NT = T // CH  # 4 t-chunks
NK = T // 128  # 16 k-tiles
NCT = C // 128  # 8 contraction tiles
N_CORES = 8
PIPE = 3  # scores->PV pipeline depth in j-steps
# HAM's 4096-cycle activity window is free-running: a burst only guarantees
# covering one full window if it spans ~2 windows (~6.8us). 64 cold N=128
# matmuls ~= 6.9us solid.
N_WARM = 64

_CACHE = {}


def _build():
    import concourse.tile as tile
    from concourse import bacc, mybir

    F32 = mybir.dt.float32
    F32R = mybir.dt.float32r
    BF16 = mybir.dt.bfloat16
    EXP = mybir.ActivationFunctionType.Exp
    IS_GE = mybir.AluOpType.is_ge

    nc = bacc.Bacc(
        "TRN2",
        target_bir_lowering=False,
        debug=False,
        enable_asserts=False,
        num_devices=N_CORES,
        num_swdge_queues=4,
    )
    xT = nc.dram_tensor("xT", [C, T], BF16, kind="ExternalInput").ap()
    wqkv = nc.dram_tensor("wqkv", [C, 384], BF16, kind="ExternalInput").ap()
    projT = nc.dram_tensor("projT", [128, C], BF16, kind="ExternalInput").ap()
    identb = nc.dram_tensor("identb", [128, 128], BF16, kind="ExternalInput").ap()
    normones = nc.dram_tensor("normones", [128, 128], F32R, kind="ExternalInput").ap()
    bias = nc.dram_tensor("bias", [128, 3], F32, kind="ExternalInput").ap()
    # output as contiguous [chunk, o-tile, 128, 512] bf16 tiles: each store is
    # one fully-contiguous 128KB DMA
    out = nc.dram_tensor("out", [NT, 8, 128, CH], BF16, kind="ExternalOutput").ap()

    with tile.TileContext(nc) as tc:
        with (
            tc.tile_pool(name="big", bufs=1) as big,
            tc.tile_pool(name="expw", bufs=8) as expw_pool,
            tc.tile_pool(name="outev", bufs=3) as outev_pool,
            tc.tile_pool(name="ps", bufs=1, space="PSUM") as ps,
        ):
            # ---- resident SBUF tensors -------------------------------------
            x_sb = big.tile([128, NCT, T], BF16, name="x_sb")
            w_sb = big.tile([128, NCT, 384], BF16, name="w_sb")
            proj_sb = big.tile([128, C], BF16, name="proj_sb")
            qT_sb = big.tile([128, T], BF16, name="qT_sb")
            kT_sb = big.tile([128, T], BF16, name="kT_sb")
            vT_sb = big.tile([128, T], BF16, name="vT_sb")
            v_aug0 = big.tile([128, NK, 65], BF16, name="v_aug0")
            v_aug1 = big.tile([128, NK, 65], BF16, name="v_aug1")
            attn_pair = big.tile([128, T], BF16, name="attn_pair")
            attn1_tmp = big.tile([64, NT, CH], BF16, name="attn1_tmp")
            ident_sb = big.tile([128, 128], BF16, name="ident_sb")
            ones_nrm = big.tile([128, 128], F32R, name="ones_nrm")
            sums_sb = big.tile([128, NT, 2, CH], F32R, name="sums_sb")
            inv_sb = big.tile([64, 2, CH], F32, name="inv_sb")
            bias_sb = big.tile([128, 3], F32, name="bias_sb")
            scr_sb = big.tile([1, 2], F32, name="scr_sb")

            # warm matmuls must not depend on any DMA (the dynamic DMA queues
            # only start delivering ~8.5us in): feed them a memset tile so the
            # solid warm block starts right when the engines come up.
            warm_sb = big.tile([128, 128], BF16, name="warm_sb")
            nc.vector.memset(warm_sb, 0.5)
            nc.sync.dma_start(out=ident_sb, in_=identb)
            nc.sync.dma_start(out=bias_sb, in_=bias)
            warm_ps = ps.tile([128, CH], F32, tag="m", bufs=2, name="warm_ps")

            def warm(n):
                if "nowarm" in _DBG:
                    return
                for _ in range(n):
                    nc.tensor.matmul(
                        warm_ps[:, 0:128], warm_sb, warm_sb, start=True, stop=True
                    )

            # preload the exp activation table during the DMA ramp (warm_sb,
            # not ident_sb: no DMA dependency)
            nc.scalar.activation(
                out=scr_sb[0:1, 0:1], in_=warm_sb[0:1, 0:1], func=EXP
            )

            # x is loaded in column halves: sweep 1 (q/k chunks 0,1) and
            # v0/v1 only touch cols 0:1024, so the critical input load is
            # w + half of x. Three queues (sync/scalar HWDGE, gpsimd SWDGE).
            # First-half x comes in 512-col pieces so the sweep's half-groups
            # (3 matmuls, ~640ns) chase the DMA stream at matching grain.
            for ct in range(NCT):
                nc.sync.dma_start(
                    out=w_sb[:, ct, :], in_=wqkv[128 * ct : 128 * ct + 128, :]
                )
                xeng = nc.scalar if ct < 4 else nc.gpsimd
                for h in (0, 1):
                    xeng.dma_start(
                        out=x_sb[:, ct, 512 * h : 512 * h + 512],
                        in_=xT[128 * ct : 128 * ct + 128, 512 * h : 512 * h + 512],
                    )
            for ct in range(NCT):
                xeng = (nc.sync, nc.scalar, nc.gpsimd)[(0, 0, 0, 0, 1, 1, 2, 2)[ct]]
                xeng.dma_start(
                    out=x_sb[:, ct, 1024:2048],
                    in_=xT[128 * ct : 128 * ct + 128, 1024:2048],
                )
            nc.sync.dma_start(out=proj_sb, in_=projT)

            nc.sync.dma_start(out=ones_nrm, in_=normones)
            nc.vector.memset(v_aug0[:, :, 64:65], 1.0)
            nc.vector.memset(v_aug1[:, :, 64:65], 1.0)

            # ---- stage B: q/k projections ----------------------------------
            # Sweep 1 (q/k for chunks 0,1) runs up front, chasing the x DMAs;
            # sweep 2 is emitted in halves inside chunk 1's j-loop as PE
            # filler (each half holds only one wide s slot so the scores/exp
            # pipeline keeps the other).
            def qk_part(gs, part, cts, evac=False):
                # 1-bank m-tag groups so the scores/exp pipeline keeps both
                # wide s slots; emitted a few ct-steps per j as PE filler.
                cols = slice(128 * part, 128 * part + 128)
                for ct in cts:
                    for c, g in gs.items():
                        nc.tensor.matmul(
                            g,
                            w_sb[:, ct, cols],
                            x_sb[:, ct, CH * c : CH * c + CH],
                            start=(ct == 0),
                            stop=(ct == NCT - 1),
                        )
                if evac:
                    dest = qT_sb if part == 0 else kT_sb
                    for c, g in gs.items():
                        nc.vector.tensor_scalar_add(
                            dest[:, CH * c : CH * c + CH],
                            g,
                            bias_sb[:, part : part + 1],
                        )

            def qk_groups(tag_name):
                return {
                    c: ps.tile([128, CH], F32, tag="m", bufs=2, name=f"{tag_name}{c}")
                    for c in (2, 3)
                }

            # sweep 1: interleave q and k groups across both s slots, ct-outer
            # so the matmuls chase the x DMA stream. The warmup block is one
            # solid ~6.9us burst: HAM's free-running activity window needs a
            # fully-covered busy window, which a burst only guarantees when it
            # spans ~two windows.
            warm(N_WARM)
            grp = {}
            for part in (0, 1):
                g = ps.tile([128, 2, CH], F32, tag="s", bufs=2, name=f"qkps_{part}")
                for ci in (0, 1):
                    grp[(part, ci)] = g[:, ci, :]
            # v chunks 0 and 1 also read only x cols 0:1024, so their matmuls
            # ride along in the ct chase: 6 matmuls (~1.3us) per arriving x
            # tile keeps the PE dense (and HAM warm) through the input phase.
            v0_ps = ps.tile([128, CH], F32, tag="m", bufs=2, name="v0_ps")
            v1_ps = ps.tile([128, CH], F32, tag="m", bufs=2, name="v1_ps")
            for ct in range(NCT):
                # per arriving 512-col x piece: q, k, and v matmuls for that
                # chunk (3 matmuls ~640ns) — half-group grain matches the DMA
                # piece grain so the PE chases the stream nearly gaplessly.
                for c, v_ps in ((0, v0_ps), (1, v1_ps)):
                    for part in (0, 1):
                        nc.tensor.matmul(
                            grp[(part, c)],
                            w_sb[:, ct, 128 * part : 128 * part + 128],
                            x_sb[:, ct, CH * c : CH * c + CH],
                            start=(ct == 0),
                            stop=(ct == NCT - 1),
                        )
                    nc.tensor.matmul(
                        v_ps,
                        w_sb[:, ct, 256:384],
                        x_sb[:, ct, CH * c : CH * c + CH],
                        start=(ct == 0),
                        stop=(ct == NCT - 1),
                    )
                    if ct < NCT - 1 or c == 0:
                        warm(2)
            for (part, c), g in grp.items():
                dest = qT_sb if part == 0 else kT_sb
                nc.vector.tensor_scalar_add(
                    dest[:, CH * c : CH * c + CH], g, bias_sb[:, part : part + 1]
                )
            for c, v_ps in ((0, v0_ps), (1, v1_ps)):
                nc.vector.tensor_scalar_add(
                    vT_sb[:, CH * c : CH * c + CH], v_ps, bias_sb[:, 2:3]
                )

            # v projection + PE transposes, per chunk; chunks 0/1 up front,
            # the rest emitted as PE filler inside stage D's j-loops.
            def emit_v_chunk(c):
                v_ps = ps.tile([128, CH], F32, tag="m", bufs=2, name=f"vps_{c}")
                for ct in range(NCT):
                    nc.tensor.matmul(
                        v_ps,
                        w_sb[:, ct, 256:384],
                        x_sb[:, ct, CH * c : CH * c + CH],
                        start=(ct == 0),
                        stop=(ct == NCT - 1),
                    )
                nc.vector.tensor_scalar_add(
                    vT_sb[:, CH * c : CH * c + CH], v_ps, bias_sb[:, 2:3]
                )

            def transposes_for(c, half=None):
                # one 128x128 transpose per k-tile covers both heads:
                # out cols 0-63 = head-0 dims, 64-127 = head-1 dims.
                kts = range(4 * c, 4 * c + 4)
                if half is not None:
                    kts = kts[2 * half : 2 * half + 2]
                for kt in kts:
                    if "notr128" in _DBG:
                        for h, v_aug in ((0, v_aug0), (1, v_aug1)):
                            hrow = slice(64 * h, 64 * h + 64)
                            tr_ps = ps.tile(
                                [128, 64], BF16, tag="m", bufs=2, name=f"tr_{h}_{kt}"
                            )
                            nc.tensor.transpose(
                                tr_ps,
                                vT_sb[hrow, 128 * kt : 128 * kt + 128],
                                ident_sb[hrow, 0:64],
                            )
                            nc.vector.tensor_copy(v_aug[:, kt, 0:64], tr_ps)
                        continue
                    tr_ps = ps.tile([128, 128], BF16, tag="m", bufs=2, name=f"tr_{kt}")
                    nc.tensor.transpose(
                        tr_ps,
                        vT_sb[:, 128 * kt : 128 * kt + 128],
                        ident_sb,
                    )
                    nc.vector.tensor_copy(v_aug0[:, kt, 0:64], tr_ps[:, 0:64])
                    nc.vector.tensor_copy(v_aug1[:, kt, 0:64], tr_ps[:, 64:128])

            transposes_for(0)

            # ---- stages D+E per t-chunk ------------------------------------
            # Each chunk's norm is emitted at its own end; its projection
            # tiles are deferred and spread through the next chunk's j-loop.
            # cross-chunk projection queue: (pc, m, epoch). Tiles are pushed
            # after their chunk's norm and drained ~1 per j-step of later
            # chunks, so no chunk ends with a flush that walls off the next
            # chunk's scores on the in-order PE queue.
            proj_queue = []

            def emit_norm(pvA, pvB, pc):
                # head 1 first: its normalized rows must still hop partitions
                # (DVE lanes are partition-locked, so an SBUF->SBUF DMA moves
                # them from 0:64 to 64:128); starting the h1 chain first lets
                # head 0's recip/mul overlap the shift. The shift rides the
                # sync HW queue — the gpsimd SW queue adds a ~2us drain.
                tcol = slice(CH * pc, CH * pc + CH)
                nc.vector.tensor_copy(sums_sb[64:65, pc, 1, :], pvB[64:65, :])
                rb1_ps = ps.tile([128, CH], F32, tag="m", bufs=2, name=f"rb1_{pc}")
                nc.tensor.matmul(
                    rb1_ps,
                    ones_nrm[64:65, :],
                    sums_sb[64:65, pc, 1, :],
                    start=True,
                    stop=True,
                )
                nc.vector.reciprocal_approx_fast(
                    out=inv_sb[:, 1, :], in_=rb1_ps[0:64, :]
                )
                nc.vector.tensor_mul(
                    attn1_tmp[:, pc, :], pvB[0:64, :], inv_sb[:, 1, :]
                )
                nc.sync.dma_start(
                    out=attn_pair[64:128, tcol], in_=attn1_tmp[:, pc, :]
                )
                nc.vector.tensor_copy(sums_sb[64:65, pc, 0, :], pvA[64:65, :])
                rb0_ps = ps.tile([128, CH], F32, tag="m", bufs=2, name=f"rb0_{pc}")
                nc.tensor.matmul(
                    rb0_ps,
                    ones_nrm[64:65, :],
                    sums_sb[64:65, pc, 0, :],
                    start=True,
                    stop=True,
                )
                nc.vector.reciprocal_approx_fast(
                    out=inv_sb[:, 0, :], in_=rb0_ps[0:64, :]
                )
                nc.vector.tensor_mul(
                    attn_pair[0:64, tcol], pvA[0:64, :], inv_sb[:, 0, :]
                )

            def emit_proj_tile(pc, m, cast_eng=None, store_eng=None):
                tcol = slice(CH * pc, CH * pc + CH)
                pr_ps = ps.tile([128, CH], F32, tag="m", bufs=2, name=f"pr_{m}_{pc}")
                nc.tensor.matmul(
                    pr_ps,
                    proj_sb[:, 128 * m : 128 * m + 128],
                    attn_pair[:, tcol],
                    start=True,
                    stop=True,
                )
                ob = outev_pool.tile([128, CH], BF16, tag="outev", name=f"ob_{m}_{pc}")
                if cast_eng is nc.scalar and "noscalcast" not in _DBG:
                    nc.scalar.copy(ob, pr_ps)
                else:
                    nc.vector.tensor_copy(ob, pr_ps)
                (store_eng or nc.sync).dma_start(out=out[pc, m], in_=ob)

            # Chunk order (1, 2, 0, 3): the last chunk processed is the
            # longest one, so both PE (scores+pv+proj filler) and ScalarE
            # (16 exps) stay dense right up to the tail — HAM never sees an
            # idle window until the final norm+proj. The short chunk 0 sits
            # mid-kernel where chunk 2's projection tiles pack its PE slack.
            chunk_order = (1, 2, 0, 3)
            for f in _DBG:
                if f.startswith("c") and f[1:].isdigit():
                    chunk_order = (1, 2, 0, 3)[: int(f[1:])]
            for ci, c in enumerate(chunk_order):
                nj = 4 * c + 4
                pvA = ps.tile([128, CH], F32, tag="pv", bufs=2, name=f"pvA_{c}")
                pvB = ps.tile([128, CH], F32, tag="pv", bufs=2, name=f"pvB_{c}")
                pending = []

                def emit_pv(item, last):
                    pj, pw, plo = item
                    # both v_aug tiles carry a ones column at index 64, so
                    # each head's denominator lands at row 64 of its pv bank
                    # with no extra matmul.
                    nc.tensor.matmul(
                        pvA[0:65, plo:CH],
                        v_aug0[:, pj, :],
                        pw[:, 0, plo:CH],
                        start=(pj == 0),
                        stop=last,
                    )
                    nc.tensor.matmul(
                        pvB[0:65, plo:CH],
                        v_aug1[:, pj, :],
                        pw[:, 1, plo:CH],
                        start=(pj == 0),
                        stop=last,
                    )

                for j in range(nj):
                    if "nowides" in _DBG:
                        sa = ps.tile([128, CH], F32, tag="s", bufs=4, name=f"sa_{c}_{j}")
                        sb = ps.tile([128, CH], F32, tag="s", bufs=4, name=f"sb_{c}_{j}")
                        s_ps = _FakeWide((sa, sb))
                    else:
                        s_ps = ps.tile(
                            [128, 2, CH], F32, tag="s", bufs=2, name=f"s_{c}_{j}"
                        )
                    # diagonal tiles: columns < 128*diag are fully masked
                    # downstream, so don't compute their scores either
                    diag = j - 4 * c
                    slo = max(0, 128 * diag)
                    for h in (0, 1):
                        hrow = slice(64 * h, 64 * h + 64)
                        nc.tensor.matmul(
                            s_ps[:, h, slo:CH],
                            kT_sb[hrow, 128 * j : 128 * j + 128],
                            qT_sb[hrow, CH * c + slo : CH * c + CH],
                            start=True,
                            stop=True,
                        )
                    w_t = expw_pool.tile(
                        [128, 2, CH], BF16, tag="expw", name=f"w_{c}_{j}"
                    )
                    # one wide exp per j covers both heads (2 PSUM banks)
                    if "nowide" in _DBG or "nowides" in _DBG:
                        for h in (0, 1):
                            nc.scalar.activation(
                                out=w_t[:, h, slo:CH], in_=s_ps[:, h, slo:CH], func=EXP
                            )
                    else:
                        nc.scalar.activation(
                            out=w_t[:, :, slo:CH], in_=s_ps[:, :, slo:CH], func=EXP
                        )
                    if diag >= 0:
                        # keep exp(score) where t >= k: within the kept column
                        # range f' = f - 128*diag, so keep f' - p >= 0
                        for h in (0, 1):
                            nc.gpsimd.affine_select(
                                out=w_t[:, h, slo:CH],
                                in_=w_t[:, h, slo:CH],
                                pattern=[[1, CH - slo]],
                                compare_op=IS_GE,
                                fill=0.0,
                                base=0,
                                channel_multiplier=-1,
                            )
                    pending.append((j, w_t, slo))
                    if c == 1:
                        # sweep 2 (q/k chunks 2,3) spread at 2 ct/step (4
                        # matmuls, ~850ns) across all 8 steps: a coarser
                        # burst between two scores emissions starves ScalarE
                        # of its next exp for the burst's whole duration.
                        if j == 0:
                            qk2q = qk_groups("qk2q")
                        if j < 4:
                            qk_part(qk2q, 0, range(2 * j, 2 * j + 2), evac=(j == 3))
                        if j == 4:
                            qk2k = qk_groups("qk2k")
                        if j >= 4:
                            qk_part(
                                qk2k, 1, range(2 * (j - 4), 2 * (j - 4) + 2),
                                evac=(j == 7),
                            )
                    if c == 1 and j in (4, 5):
                        # chunk 1's own PVs for k-tiles 4-7 need these; two
                        # transposes per step keeps the burst under ~700ns
                        transposes_for(1, half=j - 4)
                    if j == 1 and c == 2:
                        emit_v_chunk(2)
                    if j == 2 and c == 2:
                        transposes_for(2)
                    if j == 1 and c == 3:
                        emit_v_chunk(3)
                    if j == 3 and c == 3:
                        transposes_for(3)
                    while len(pending) > PIPE:
                        item, pending = pending[0], pending[1:]
                        emit_pv(item, last=False)
                    # drain the proj queue ~1 tile/step. A tile pushed at the
                    # immediately-previous boundary waits until j>=3 so its
                    # matmul never gates on the still-running norm chain
                    # (an unmet attn_pair dep stalls the in-order PE queue).
                    took = 0
                    while proj_queue and took < (2 if len(proj_queue) > nj - j else 1):
                        pc, m, epoch = proj_queue[0]
                        if epoch == ci - 1 and j < 2:
                            break
                        proj_queue.pop(0)
                        ce = nc.scalar if (c == 0 and m % 2) else None
                        emit_proj_tile(pc, m, cast_eng=ce)
                        took += 1
                while pending:
                    item, pending = pending[0], pending[1:]
                    emit_pv(item, last=(len(pending) == 0))
                # norm as early as its inputs allow (right after the final
                # stop=True PV); this chunk's proj tiles queue up behind it.
                emit_norm(pvA, pvB, c)
                proj_queue.extend((c, m, ci) for m in range(8))

            # tail: whatever projection tiles remain (the final chunk's 8);
            # casts and stores alternate Scalar/Vector + sync/scalar queues
            # (ScalarE is idle once the exps are done)
            if "notail" not in _DBG:
                if "nokeep" not in _DBG:
                    # HAM keepalive through the tail: dummy matmuls into a
                    # fresh s-slot (no tail readers of the s banks)
                    tw = ps.tile([128, 2, CH], F32, tag="s", bufs=2, name="tailwarm")
                for i, (pc, m, epoch) in enumerate(proj_queue):
                    emit_proj_tile(
                        pc,
                        m,
                        cast_eng=nc.scalar if i % 2 else None,
                        store_eng=nc.scalar if i % 2 == 0 else None,
                    )
                    if "nokeep" not in _DBG:
                        for _ in range(2):
                            nc.tensor.matmul(
                                tw[:, 0, 0:128],
                                warm_sb,
                                warm_sb,
                                start=True,
                                stop=True,
                            )
                proj_queue.clear()

    nc.compile()
    return nc


def _get_nc():
    if "nc" not in _CACHE:
        _CACHE["nc"] = _build()
    return _CACHE["nc"]


def _make_in_maps(x, wqkv_w, wqkv_b, proj_w):
    bf = ml_dtypes.bfloat16
    xT = np.ascontiguousarray(np.asarray(x, np.float32).T.astype(bf))
    identb = np.eye(128, dtype=bf)
    scale = np.float32(1.0 / np.sqrt(C))
    in_maps = []
    for i in range(N_CORES):
        rows = []
        biases = []
        for blk, s in ((0, scale), (1, None), (2, None)):
            sl = slice(blk * C + 128 * i, blk * C + 128 * i + 128)
            w = np.asarray(wqkv_w[sl], np.float32)
            b = np.asarray(wqkv_b[sl], np.float32)
            if s is not None:
                w = w * s
                b = b * s
            rows.append(w)
            biases.append(b)
        W = np.concatenate(rows, axis=0)  # [384, 1024]
        B = np.stack(biases, axis=1)  # [128, 3]
        pT = np.asarray(proj_w[:, 128 * i : 128 * i + 128], np.float32).T  # [128, 1024]
        normones = np.zeros((128, 128), np.float32)
        normones[0] = 1.0
        normones[64] = 1.0
        in_maps.append(
            {
                "xT": xT,
                "wqkv": np.ascontiguousarray(W.T.astype(bf)),
                "projT": np.ascontiguousarray(pT.astype(bf)),
                "identb": identb,
                "normones": normones,
                "bias": np.ascontiguousarray(B),
            }
        )
    return in_maps


def kernel(x, wqkv_w, wqkv_b, proj_w, proj_b, _trace=False, _tmpdir=None):
    from concourse.bass_utils import run_bass_kernel_spmd

    nc = _get_nc()
    in_maps = _make_in_maps(x, wqkv_w, wqkv_b, proj_w)
    res = run_bass_kernel_spmd(
        nc,
        in_maps,
        core_ids=list(range(N_CORES)),
        trace=_trace,
        tmpdir=_tmpdir,
    )
    acc = np.zeros((NT, 8, 128, CH), np.float64)
    for rmap in res.results:
        acc += rmap["out"].astype(np.float64)
    partialT = acc.transpose(1, 2, 0, 3).reshape(C, T)  # [o, t]
    full = partialT.T + np.asarray(proj_b, np.float64)[None, :]
    if _trace:
        _CACHE["last_result"] = res
    return full.astype(np.float32)



# revision 44
# speedup vs baseline: 1.0229x; 1.0229x over previous
"""Causal self-attention (T=2048, C=1024, H=16) on 8 trn2 NeuronCores.

Tensor-parallel over heads: core i computes heads 2i, 2i+1 (q/k/v rows
128i:128i+128 of each 1024-row block of wqkv_w, proj_w columns
128i:128i+128), producing a partial output projection; partials are summed
on the host (the all-reduce of the sharding hint).

Per-core Bass/Tile kernel, bf16 matmuls with fp32 PSUM accumulation.
Layout puts head 0's attention pipeline on partitions 0-63 and head 1's on
64-127 so the two heads' K=64 matmuls land in disjoint PE row groups
(auto tile_position from base partitions) and run concurrently:
  B. warmup matmuls on the identity tile keep HAM's activity window busy
     while the x DMAs stream, so stage B starts at K=8/8 (2.4GHz).
     qkvT[j, t] = wqkv.T @ xT, contraction-tile outer so matmuls chase the
     x DMAs; q rows pre-scaled by 1/sqrt(C) on the host. v's 128x128 PE
     transposes produce both heads' v_aug tiles at once and are interleaved
     with the v matmuls so they never form a transpose-only PE window.
  D. per 512-col t-chunk: sT[k, 2, t] = kT.T @ qT (both heads, one wide
     2-bank PSUM tile) -> ONE wide exp per j on ScalarE (bf16 out, no
     max-subtraction: |scores| < ~1) -> causal affine_select on gpsimd
     (diagonal k-tiles only, both heads in one 3D op) -> PV:
       pvA[0:65]  += v_aug0.T @ w0   (M=65, ones col = head-0 denominator)
       den1[96:97]+= ones.T   @ w1   (M=1 packed into PE col group 3,
                                      concurrent with the pvA matmul)
       pvB[64:128]+= v_aug1.T @ w1   (M=64 at base partition 64)
     Normalize on DVE: denominators broadcast via two concurrent K=1
     matmuls (row groups 2/3), reciprocal_approx_fast, one multiply per
     head -> attn_pair[128, T] bf16 (h0 rows 0-63, h1 rows 64-127).
     Previous chunk's normalize/proj spread through the j-loop as PE
     filler while ScalarE (exp) is the per-step bottleneck.
  E. partialT[o, t] = projT.T @ attn_pair: per o-tile, two concurrent
     K=64 matmuls (row groups {2,3} then {0,1}; the rows-64-127 one is
     emitted first / start=True because its drain path is shorter).
"""

import sys

if "/opt/trn_rl_repo" not in sys.path:
    sys.path.insert(0, "/opt/trn_rl_repo")

import os

import ml_dtypes
import numpy as np

_DBG = set(os.environ.get("KDBG", "").split(","))


class _FakeWide:
    """Pair of 1-bank PSUM tiles indexed like a [128, 2, CH] wide tile."""

    def __init__(self, tiles):
        self.tiles = tiles

    def __getitem__(self, idx):
        assert len(idx) == 3
        return self.tiles[idx[1]][idx[0], idx[2]]

T = 2048
C = 1024
CH = 512  # t-chunk width (one PSUM bank of fp32)
NT = T // CH  # 4 t-chunks
NK = T // 128  # 16 k-tiles
NCT = C // 128  # 8 contraction tiles
N_CORES = 8
PIPE = 3  # scores->PV pipeline depth in j-steps
# HAM's 4096-cycle activity window is free-running: a burst only guarantees
# covering one full window if it spans ~2 windows (~6.8us). 64 cold N=128
# matmuls ~= 6.9us solid.
N_WARM = 64

_CACHE = {}


def _build():
    import concourse.tile as tile
    from concourse import bacc, mybir

    F32 = mybir.dt.float32
    F32R = mybir.dt.float32r
    BF16 = mybir.dt.bfloat16
    EXP = mybir.ActivationFunctionType.Exp
    IS_GE = mybir.AluOpType.is_ge

    nc = bacc.Bacc(
        "TRN2",
        target_bir_lowering=False,
        debug=False,
        enable_asserts=False,
        num_devices=N_CORES,
        num_swdge_queues=4,
    )
    xT = nc.dram_tensor("xT", [C, T], BF16, kind="ExternalInput").ap()
    wqkv = nc.dram_tensor("wqkv", [C, 384], BF16, kind="ExternalInput").ap()
    projT = nc.dram_tensor("projT", [128, C], BF16, kind="ExternalInput").ap()
    identb = nc.dram_tensor("identb", [128, 128], BF16, kind="ExternalInput").ap()
    normones = nc.dram_tensor("normones", [128, 128], F32R, kind="ExternalInput").ap()
    bias = nc.dram_tensor("bias", [128, 3], F32, kind="ExternalInput").ap()
    # output as contiguous [chunk, o-tile, 128, 512] bf16 tiles: each store is
    # one fully-contiguous 128KB DMA
    out = nc.dram_tensor("out", [NT, 8, 128, CH], BF16, kind="ExternalOutput").ap()

    with tile.TileContext(nc) as tc:
        with (
            tc.tile_pool(name="big", bufs=1) as big,
            tc.tile_pool(name="expw", bufs=8) as expw_pool,
            tc.tile_pool(name="outev", bufs=3) as outev_pool,
            tc.tile_pool(name="ps", bufs=1, space="PSUM") as ps,
        ):
            # ---- resident SBUF tensors -------------------------------------
            x_sb = big.tile([128, NCT, T], BF16, name="x_sb")
            w_sb = big.tile([128, NCT, 384], BF16, name="w_sb")
            proj_sb = big.tile([128, C], BF16, name="proj_sb")
            qT_sb = big.tile([128, T], BF16, name="qT_sb")
            kT_sb = big.tile([128, T], BF16, name="kT_sb")
            vT_sb = big.tile([128, T], BF16, name="vT_sb")
            v_aug0 = big.tile([128, NK, 65], BF16, name="v_aug0")
            v_aug1 = big.tile([128, NK, 65], BF16, name="v_aug1")
            attn_pair = big.tile([128, T], BF16, name="attn_pair")
            attn1_tmp = big.tile([64, NT, CH], BF16, name="attn1_tmp")
            ident_sb = big.tile([128, 128], BF16, name="ident_sb")
            ones_nrm = big.tile([128, 128], F32R, name="ones_nrm")
            sums_sb = big.tile([128, NT, 2, CH], F32R, name="sums_sb")
            inv_sb = big.tile([64, 2, CH], F32, name="inv_sb")
            bias_sb = big.tile([128, 3], F32, name="bias_sb")
            scr_sb = big.tile([1, 2], F32, name="scr_sb")

            # warm matmuls must not depend on any DMA (the dynamic DMA queues
            # only start delivering ~8.5us in): feed them a memset tile so the
            # solid warm block starts right when the engines come up.
            warm_sb = big.tile([128, 128], BF16, name="warm_sb")
            nc.vector.memset(warm_sb, 0.5)
            nc.sync.dma_start(out=ident_sb, in_=identb)
            nc.sync.dma_start(out=bias_sb, in_=bias)
            warm_ps = ps.tile([128, CH], F32, tag="m", bufs=2, name="warm_ps")

            def warm(n):
                if "nowarm" in _DBG:
                    return
                for _ in range(n):
                    nc.tensor.matmul(
                        warm_ps[:, 0:128], warm_sb, warm_sb, start=True, stop=True
                    )

            # preload the exp activation table during the DMA ramp (warm_sb,
            # not ident_sb: no DMA dependency)
            nc.scalar.activation(
                out=scr_sb[0:1, 0:1], in_=warm_sb[0:1, 0:1], func=EXP
            )

            # x is loaded in column halves: sweep 1 (q/k chunks 0,1) and
            # v0/v1 only touch cols 0:1024, so the critical input load is
            # w + half of x. Three queues (sync/scalar HWDGE, gpsimd SWDGE).
            # First-half x comes in 512-col pieces so the sweep's half-groups
            # (3 matmuls, ~640ns) chase the DMA stream at matching grain.
            for ct in range(NCT):
                nc.sync.dma_start(
                    out=w_sb[:, ct, :], in_=wqkv[128 * ct : 128 * ct + 128, :]
                )
                xeng = nc.scalar if ct < 4 else nc.gpsimd
                for h in (0, 1):
                    xeng.dma_start(
                        out=x_sb[:, ct, 512 * h : 512 * h + 512],
                        in_=xT[128 * ct : 128 * ct + 128, 512 * h : 512 * h + 512],
                    )
            for ct in range(NCT):
                xeng = (nc.sync, nc.scalar, nc.gpsimd)[(0, 0, 0, 0, 1, 1, 2, 2)[ct]]
                xeng.dma_start(
                    out=x_sb[:, ct, 1024:2048],
                    in_=xT[128 * ct : 128 * ct + 128, 1024:2048],
                )
            nc.sync.dma_start(out=proj_sb, in_=projT)

            nc.sync.dma_start(out=ones_nrm, in_=normones)
            nc.vector.memset(v_aug0[:, :, 64:65], 1.0)
            nc.vector.memset(v_aug1[:, :, 64:65], 1.0)

            # ---- stage B: q/k projections ----------------------------------
            # Sweep 1 (q/k for chunks 0,1) runs up front, chasing the x DMAs;
            # sweep 2 is emitted in halves inside chunk 1's j-loop as PE
            # filler (each half holds only one wide s slot so the scores/exp
            # pipeline keeps the other).
            def qk_part(gs, part, cts, evac=False):
                # 1-bank m-tag groups so the scores/exp pipeline keeps both
                # wide s slots; emitted a few ct-steps per j as PE filler.
                cols = slice(128 * part, 128 * part + 128)
                for ct in cts:
                    for c, g in gs.items():
                        nc.tensor.matmul(
                            g,
                            w_sb[:, ct, cols],
                            x_sb[:, ct, CH * c : CH * c + CH],
                            start=(ct == 0),
                            stop=(ct == NCT - 1),
                        )
                if evac:
                    dest = qT_sb if part == 0 else kT_sb
                    for c, g in gs.items():
                        nc.vector.tensor_scalar_add(
                            dest[:, CH * c : CH * c + CH],
                            g,
                            bias_sb[:, part : part + 1],
                        )

            def qk_groups(tag_name):
                return {
                    c: ps.tile([128, CH], F32, tag="m", bufs=2, name=f"{tag_name}{c}")
                    for c in (2, 3)
                }

            # sweep 1: interleave q and k groups across both s slots, ct-outer
            # so the matmuls chase the x DMA stream. The warmup block is one
            # solid ~6.9us burst: HAM's free-running activity window needs a
            # fully-covered busy window, which a burst only guarantees when it
            # spans ~two windows.
            warm(N_WARM)
            grp = {}
            for part in (0, 1):
                g = ps.tile([128, 2, CH], F32, tag="s", bufs=2, name=f"qkps_{part}")
                for ci in (0, 1):
                    grp[(part, ci)] = g[:, ci, :]
            # v chunks 0 and 1 also read only x cols 0:1024, so their matmuls
            # ride along in the ct chase: 6 matmuls (~1.3us) per arriving x
            # tile keeps the PE dense (and HAM warm) through the input phase.
            v0_ps = ps.tile([128, CH], F32, tag="m", bufs=2, name="v0_ps")
            v1_ps = ps.tile([128, CH], F32, tag="m", bufs=2, name="v1_ps")
            for ct in range(NCT):
                # per arriving 512-col x piece: q, k, and v matmuls for that
                # chunk (3 matmuls ~640ns) — half-group grain matches the DMA
                # piece grain so the PE chases the stream nearly gaplessly.
                for c, v_ps in ((0, v0_ps), (1, v1_ps)):
                    for part in (0, 1):
                        nc.tensor.matmul(
                            grp[(part, c)],
                            w_sb[:, ct, 128 * part : 128 * part + 128],
                            x_sb[:, ct, CH * c : CH * c + CH],
                            start=(ct == 0),
                            stop=(ct == NCT - 1),
                        )
                    nc.tensor.matmul(
                        v_ps,
                        w_sb[:, ct, 256:384],
                        x_sb[:, ct, CH * c : CH * c + CH],
                        start=(ct == 0),
                        stop=(ct == NCT - 1),
                    )
                    if ct < NCT - 1 or c == 0:
                        warm(2)
            for (part, c), g in grp.items():
                dest = qT_sb if part == 0 else kT_sb
                nc.vector.tensor_scalar_add(
                    dest[:, CH * c : CH * c + CH], g, bias_sb[:, part : part + 1]
                )
            for c, v_ps in ((0, v0_ps), (1, v1_ps)):
                nc.vector.tensor_scalar_add(
                    vT_sb[:, CH * c : CH * c + CH], v_ps, bias_sb[:, 2:3]
                )

            # v projection + PE transposes, per chunk; chunks 0/1 up front,
            # the rest emitted as PE filler inside stage D's j-loops.
            def emit_v_chunk(c):
                v_ps = ps.tile([128, CH], F32, tag="m", bufs=2, name=f"vps_{c}")
                for ct in range(NCT):
                    nc.tensor.matmul(
                        v_ps,
                        w_sb[:, ct, 256:384],
                        x_sb[:, ct, CH * c : CH * c + CH],
                        start=(ct == 0),
                        stop=(ct == NCT - 1),
                    )
                nc.vector.tensor_scalar_add(
                    vT_sb[:, CH * c : CH * c + CH], v_ps, bias_sb[:, 2:3]
                )

            def transposes_for(c, half=None):
                # one 128x128 transpose per k-tile covers both heads:
                # out cols 0-63 = head-0 dims, 64-127 = head-1 dims.
                kts = range(4 * c, 4 * c + 4)
                if half is not None:
                    kts = kts[2 * half : 2 * half + 2]
                for kt in kts:
                    if "notr128" in _DBG:
                        for h, v_aug in ((0, v_aug0), (1, v_aug1)):
                            hrow = slice(64 * h, 64 * h + 64)
                            tr_ps = ps.tile(
                                [128, 64], BF16, tag="m", bufs=2, name=f"tr_{h}_{kt}"
                            )
                            nc.tensor.transpose(
                                tr_ps,
                                vT_sb[hrow, 128 * kt : 128 * kt + 128],
                                ident_sb[hrow, 0:64],
                            )
                            nc.vector.tensor_copy(v_aug[:, kt, 0:64], tr_ps)
                        continue
                    tr_ps = ps.tile([128, 128], BF16, tag="m", bufs=2, name=f"tr_{kt}")
                    nc.tensor.transpose(
                        tr_ps,
                        vT_sb[:, 128 * kt : 128 * kt + 128],
                        ident_sb,
                    )
                    nc.vector.tensor_copy(v_aug0[:, kt, 0:64], tr_ps[:, 0:64])
                    nc.vector.tensor_copy(v_aug1[:, kt, 0:64], tr_ps[:, 64:128])

            transposes_for(0)

            # ---- stages D+E per t-chunk ------------------------------------
            # Each chunk's norm is emitted at its own end; its projection
            # tiles are deferred and spread through the next chunk's j-loop.
            # cross-chunk projection queue: (pc, m, epoch). Tiles are pushed
            # after their chunk's norm and drained ~1 per j-step of later
            # chunks, so no chunk ends with a flush that walls off the next
            # chunk's scores on the in-order PE queue.
            proj_queue = []

            def emit_norm(pvA, pvB, pc):
                # head 1 first: its normalized rows must still hop partitions
                # (DVE lanes are partition-locked, so an SBUF->SBUF DMA moves
                # them from 0:64 to 64:128); starting the h1 chain first lets
                # head 0's recip/mul overlap the shift. The shift rides the
                # sync HW queue — the gpsimd SW queue adds a ~2us drain.
                tcol = slice(CH * pc, CH * pc + CH)
                nc.vector.tensor_copy(sums_sb[64:65, pc, 1, :], pvB[64:65, :])
                rb1_ps = ps.tile([128, CH], F32, tag="m", bufs=2, name=f"rb1_{pc}")
                nc.tensor.matmul(
                    rb1_ps,
                    ones_nrm[64:65, :],
                    sums_sb[64:65, pc, 1, :],
                    start=True,
                    stop=True,
                )
                nc.vector.reciprocal_approx_fast(
                    out=inv_sb[:, 1, :], in_=rb1_ps[0:64, :]
                )
                nc.vector.tensor_mul(
                    attn1_tmp[:, pc, :], pvB[0:64, :], inv_sb[:, 1, :]
                )
                nc.sync.dma_start(
                    out=attn_pair[64:128, tcol], in_=attn1_tmp[:, pc, :]
                )
                nc.vector.tensor_copy(sums_sb[64:65, pc, 0, :], pvA[64:65, :])
                rb0_ps = ps.tile([128, CH], F32, tag="m", bufs=2, name=f"rb0_{pc}")
                nc.tensor.matmul(
                    rb0_ps,
                    ones_nrm[64:65, :],
                    sums_sb[64:65, pc, 0, :],
                    start=True,
                    stop=True,
                )
                nc.vector.reciprocal_approx_fast(
                    out=inv_sb[:, 0, :], in_=rb0_ps[0:64, :]
                )
                nc.vector.tensor_mul(
                    attn_pair[0:64, tcol], pvA[0:64, :], inv_sb[:, 0, :]
                )

            def emit_proj_tile(pc, m, cast_eng=None, store_eng=None):
                tcol = slice(CH * pc, CH * pc + CH)
                pr_ps = ps.tile([128, CH], F32, tag="m", bufs=2, name=f"pr_{m}_{pc}")
                nc.tensor.matmul(
                    pr_ps,
                    proj_sb[:, 128 * m : 128 * m + 128],
                    attn_pair[:, tcol],
                    start=True,
                    stop=True,
                )
                ob = outev_pool.tile([128, CH], BF16, tag="outev", name=f"ob_{m}_{pc}")
                if cast_eng is nc.scalar and "noscalcast" not in _DBG:
                    nc.scalar.copy(ob, pr_ps)
                else:
                    nc.vector.tensor_copy(ob, pr_ps)
                (store_eng or nc.sync).dma_start(out=out[pc, m], in_=ob)

            # Chunk order (1, 2, 0, 3): the last chunk processed is the
            # longest one, so both PE (scores+pv+proj filler) and ScalarE
            # (16 exps) stay dense right up to the tail — HAM never sees an
            # idle window until the final norm+proj. The short chunk 0 sits
            # mid-kernel where chunk 2's projection tiles pack its PE slack.
            chunk_order = (1, 2, 0, 3)
            for f in _DBG:
                if f.startswith("c") and f[1:].isdigit():
                    chunk_order = (1, 2, 0, 3)[: int(f[1:])]
            for ci, c in enumerate(chunk_order):
                nj = 4 * c + 4
                pvA = ps.tile([128, CH], F32, tag="pv", bufs=2, name=f"pvA_{c}")
                pvB = ps.tile([128, CH], F32, tag="pv", bufs=2, name=f"pvB_{c}")
                pending = []

                def emit_pv(item, last):
                    pj, pw, plo = item
                    # both v_aug tiles carry a ones column at index 64, so
                    # each head's denominator lands at row 64 of its pv bank
                    # with no extra matmul.
                    nc.tensor.matmul(
                        pvA[0:65, plo:CH],
                        v_aug0[:, pj, :],
                        pw[:, 0, plo:CH],
                        start=(pj == 0),
                        stop=last,
                    )
                    nc.tensor.matmul(
                        pvB[0:65, plo:CH],
                        v_aug1[:, pj, :],
                        pw[:, 1, plo:CH],
                        start=(pj == 0),
                        stop=last,
                    )

                for j in range(nj):
                    if "nowides" in _DBG:
                        sa = ps.tile([128, CH], F32, tag="s", bufs=4, name=f"sa_{c}_{j}")
                        sb = ps.tile([128, CH], F32, tag="s", bufs=4, name=f"sb_{c}_{j}")
                        s_ps = _FakeWide((sa, sb))
                    else:
                        s_ps = ps.tile(
                            [128, 2, CH], F32, tag="s", bufs=2, name=f"s_{c}_{j}"
                        )
                    # diagonal tiles: columns < 128*diag are fully masked
                    # downstream, so don't compute their scores either
                    diag = j - 4 * c
                    slo = max(0, 128 * diag)
                    for h in (0, 1):
                        hrow = slice(64 * h, 64 * h + 64)
                        nc.tensor.matmul(
                            s_ps[:, h, slo:CH],
                            kT_sb[hrow, 128 * j : 128 * j + 128],
                            qT_sb[hrow, CH * c + slo : CH * c + CH],
                            start=True,
                            stop=True,
                        )
                    w_t = expw_pool.tile(
                        [128, 2, CH], BF16, tag="expw", name=f"w_{c}_{j}"
                    )
                    # one wide exp per j covers both heads (2 PSUM banks)
                    if "nowide" in _DBG or "nowides" in _DBG:
                        for h in (0, 1):
                            nc.scalar.activation(
                                out=w_t[:, h, slo:CH], in_=s_ps[:, h, slo:CH], func=EXP
                            )
                    else:
                        nc.scalar.activation(
                            out=w_t[:, :, slo:CH], in_=s_ps[:, :, slo:CH], func=EXP
                        )
                    if diag >= 0:
                        # keep exp(score) where t >= k: within the kept column
                        # range f' = f - 128*diag, so keep f' - p >= 0
                        for h in (0, 1):
                            nc.gpsimd.affine_select(
                                out=w_t[:, h, slo:CH],
                                in_=w_t[:, h, slo:CH],
                                pattern=[[1, CH - slo]],
                                compare_op=IS_GE,
                                fill=0.0,
                                base=0,
                                channel_multiplier=-1,
                            )
                    pending.append((j, w_t, slo))
                    if c == 1:
                        # sweep 2 (q/k chunks 2,3) + v1 as chunk 1's filler,
                        # a few hundred ns of PE work per j-step
                        if j == 0:
                            qk2q = qk_groups("qk2q")
                            qk_part(qk2q, 0, range(0, 4))
                        if j == 1:
                            qk_part(qk2q, 0, range(4, 8), evac=True)
                        if j == 2:
                            qk2k = qk_groups("qk2k")
                            qk_part(qk2k, 1, range(0, 4))
                        if j == 3:
                            qk_part(qk2k, 1, range(4, 8), evac=True)
                        if j == 4:
                            transposes_for(1)
                    if j == 1 and c == 2:
                        emit_v_chunk(2)
                    if j == 2 and c == 2:
                        transposes_for(2)
                    if j == 1 and c == 3:
                        emit_v_chunk(3)
                    if j == 3 and c == 3:
                        transposes_for(3)
                    while len(pending) > PIPE:
                        item, pending = pending[0], pending[1:]
                        emit_pv(item, last=False)
                    # drain the proj queue ~1 tile/step. A tile pushed at the
                    # immediately-previous boundary waits until j>=3 so its
                    # matmul never gates on the still-running norm chain
                    # (an unmet attn_pair dep stalls the in-order PE queue).
                    took = 0
                    while proj_queue and took < (2 if len(proj_queue) > nj - j else 1):
                        pc, m, epoch = proj_queue[0]
                        if epoch == ci - 1 and j < 3:
                            break
                        proj_queue.pop(0)
                        ce = nc.scalar if (c == 0 and m % 2) else None
                        emit_proj_tile(pc, m, cast_eng=ce)
                        took += 1
                while pending:
                    item, pending = pending[0], pending[1:]
                    emit_pv(item, last=(len(pending) == 0))
                # norm as early as its inputs allow (right after the final
                # stop=True PV); this chunk's proj tiles queue up behind it.
                emit_norm(pvA, pvB, c)
                proj_queue.extend((c, m, ci) for m in range(8))

            # tail: whatever projection tiles remain (the final chunk's 8);
            # casts and stores alternate Scalar/Vector + sync/scalar queues
            # (ScalarE is idle once the exps are done)
            if "notail" not in _DBG:
                if "nokeep" not in _DBG:
                    # HAM keepalive through the tail: dummy matmuls into a
                    # fresh s-slot (no tail readers of the s banks)
                    tw = ps.tile([128, 2, CH], F32, tag="s", bufs=2, name="tailwarm")
                for i, (pc, m, epoch) in enumerate(proj_queue):
                    emit_proj_tile(
                        pc,
                        m,
                        cast_eng=nc.scalar if i % 2 else None,
                        store_eng=nc.scalar if i % 2 == 0 else None,
                    )
                    if "nokeep" not in _DBG:
                        for _ in range(2):
                            nc.tensor.matmul(
                                tw[:, 0, 0:128],
                                warm_sb,
                                warm_sb,
                                start=True,
                                stop=True,
                            )
                proj_queue.clear()

    nc.compile()
    return nc


def _get_nc():
    if "nc" not in _CACHE:
        _CACHE["nc"] = _build()
    return _CACHE["nc"]


def _make_in_maps(x, wqkv_w, wqkv_b, proj_w):
    bf = ml_dtypes.bfloat16
    xT = np.ascontiguousarray(np.asarray(x, np.float32).T.astype(bf))
    identb = np.eye(128, dtype=bf)
    scale = np.float32(1.0 / np.sqrt(C))
    in_maps = []
    for i in range(N_CORES):
        rows = []
        biases = []
        for blk, s in ((0, scale), (1, None), (2, None)):
            sl = slice(blk * C + 128 * i, blk * C + 128 * i + 128)
            w = np.asarray(wqkv_w[sl], np.float32)
            b = np.asarray(wqkv_b[sl], np.float32)
            if s is not None:
                w = w * s
                b = b * s
            rows.append(w)
            biases.append(b)
        W = np.concatenate(rows, axis=0)  # [384, 1024]
        B = np.stack(biases, axis=1)  # [128, 3]
        pT = np.asarray(proj_w[:, 128 * i : 128 * i + 128], np.float32).T  # [128, 1024]
        normones = np.zeros((128, 128), np.float32)
        normones[0] = 1.0
        normones[64] = 1.0
        in_maps.append(
            {
                "xT": xT,
                "wqkv": np.ascontiguousarray(W.T.astype(bf)),
                "projT": np.ascontiguousarray(pT.astype(bf)),
                "identb": identb,
                "normones": normones,
                "bias": np.ascontiguousarray(B),
            }
        )
    return in_maps


def kernel(x, wqkv_w, wqkv_b, proj_w, proj_b, _trace=False, _tmpdir=None):
    from concourse.bass_utils import run_bass_kernel_spmd

    nc = _get_nc()
    in_maps = _make_in_maps(x, wqkv_w, wqkv_b, proj_w)
    res = run_bass_kernel_spmd(
        nc,
        in_maps,
        core_ids=list(range(N_CORES)),
        trace=_trace,
        tmpdir=_tmpdir,
    )
    acc = np.zeros((NT, 8, 128, CH), np.float64)
    for rmap in res.results:
        acc += rmap["out"].astype(np.float64)
    partialT = acc.transpose(1, 2, 0, 3).reshape(C, T)  # [o, t]
    full = partialT.T + np.asarray(proj_b, np.float64)[None, :]
    if _trace:
        _CACHE["last_result"] = res
    return full.astype(np.float32)



# revision 46
# speedup vs baseline: 1.0763x; 1.0522x over previous
"""Causal self-attention (T=2048, C=1024, H=16) on 8 trn2 NeuronCores.

Tensor-parallel over heads: core i computes heads 2i, 2i+1 (q/k/v rows
128i:128i+128 of each 1024-row block of wqkv_w, proj_w columns
128i:128i+128), producing a partial output projection; partials are summed
on the host (the all-reduce of the sharding hint).

Per-core Bass/Tile kernel, bf16 matmuls with fp32 PSUM accumulation.
Layout puts head 0's attention pipeline on partitions 0-63 and head 1's on
64-127 so the two heads' K=64 matmuls land in disjoint PE row groups
(auto tile_position from base partitions) and run concurrently:
  B. warmup matmuls on the identity tile keep HAM's activity window busy
     while the x DMAs stream, so stage B starts at K=8/8 (2.4GHz).
     qkvT[j, t] = wqkv.T @ xT, contraction-tile outer so matmuls chase the
     x DMAs; q rows pre-scaled by 1/sqrt(C) on the host. v's 128x128 PE
     transposes produce both heads' v_aug tiles at once and are interleaved
     with the v matmuls so they never form a transpose-only PE window.
  D. per 512-col t-chunk: sT[k, 2, t] = kT.T @ qT (both heads, one wide
     2-bank PSUM tile) -> ONE wide exp per j on ScalarE (bf16 out, no
     max-subtraction: |scores| < ~1) -> causal affine_select on gpsimd
     (diagonal k-tiles only, both heads in one 3D op) -> PV:
       pvA[0:65]  += v_aug0.T @ w0   (M=65, ones col = head-0 denominator)
       den1[96:97]+= ones.T   @ w1   (M=1 packed into PE col group 3,
                                      concurrent with the pvA matmul)
       pvB[64:128]+= v_aug1.T @ w1   (M=64 at base partition 64)
     Normalize on DVE: denominators broadcast via two concurrent K=1
     matmuls (row groups 2/3), reciprocal_approx_fast, one multiply per
     head -> attn_pair[128, T] bf16 (h0 rows 0-63, h1 rows 64-127).
     Previous chunk's normalize/proj spread through the j-loop as PE
     filler while ScalarE (exp) is the per-step bottleneck.
  E. partialT[o, t] = projT.T @ attn_pair: per o-tile, two concurrent
     K=64 matmuls (row groups {2,3} then {0,1}; the rows-64-127 one is
     emitted first / start=True because its drain path is shorter).
"""

import sys

if "/opt/trn_rl_repo" not in sys.path:
    sys.path.insert(0, "/opt/trn_rl_repo")

import os

import ml_dtypes
import numpy as np

_DBG = set(os.environ.get("KDBG", "").split(","))


class _FakeWide:
    """Pair of 1-bank PSUM tiles indexed like a [128, 2, CH] wide tile."""

    def __init__(self, tiles):
        self.tiles = tiles

    def __getitem__(self, idx):
        assert len(idx) == 3
        return self.tiles[idx[1]][idx[0], idx[2]]

T = 2048
C = 1024
CH = 512  # t-chunk width (one PSUM bank of fp32)
NT = T // CH  # 4 t-chunks
NK = T // 128  # 16 k-tiles
NCT = C // 128  # 8 contraction tiles
N_CORES = 8
PIPE = 3  # scores->PV pipeline depth in j-steps
# HAM's 4096-cycle activity window is free-running: a burst only guarantees
# covering one full window if it spans ~2 windows (~6.8us). 64 cold N=128
# matmuls ~= 6.9us solid.
N_WARM = 64

_CACHE = {}


def _build():
    import concourse.tile as tile
    from concourse import bacc, mybir

    F32 = mybir.dt.float32
    F32R = mybir.dt.float32r
    BF16 = mybir.dt.bfloat16
    EXP = mybir.ActivationFunctionType.Exp
    IS_GE = mybir.AluOpType.is_ge

    nc = bacc.Bacc(
        "TRN2",
        target_bir_lowering=False,
        debug=False,
        enable_asserts=False,
        num_devices=N_CORES,
        num_swdge_queues=4,
    )
    xT = nc.dram_tensor("xT", [C, T], BF16, kind="ExternalInput").ap()
    wqkv = nc.dram_tensor("wqkv", [C, 384], BF16, kind="ExternalInput").ap()
    projT = nc.dram_tensor("projT", [128, C], BF16, kind="ExternalInput").ap()
    identb = nc.dram_tensor("identb", [128, 128], BF16, kind="ExternalInput").ap()
    normones = nc.dram_tensor("normones", [128, 128], F32R, kind="ExternalInput").ap()
    bias = nc.dram_tensor("bias", [128, 3], F32, kind="ExternalInput").ap()
    # output as contiguous [chunk, o-tile, 128, 512] bf16 tiles: each store is
    # one fully-contiguous 128KB DMA
    out = nc.dram_tensor("out", [NT, 8, 128, CH], BF16, kind="ExternalOutput").ap()

    with tile.TileContext(nc) as tc:
        with (
            tc.tile_pool(name="big", bufs=1) as big,
            tc.tile_pool(name="expw", bufs=8) as expw_pool,
            tc.tile_pool(name="outev", bufs=6) as outev_pool,
            tc.tile_pool(name="ps", bufs=1, space="PSUM") as ps,
        ):
            # ---- resident SBUF tensors -------------------------------------
            x_sb = big.tile([128, NCT, T], BF16, name="x_sb")
            w_sb = big.tile([128, NCT, 384], BF16, name="w_sb")
            proj_sb = big.tile([128, C], BF16, name="proj_sb")
            qT_sb = big.tile([128, T], BF16, name="qT_sb")
            kT_sb = big.tile([128, T], BF16, name="kT_sb")
            vT_sb = big.tile([128, T], BF16, name="vT_sb")
            v_aug0 = big.tile([128, NK, 65], BF16, name="v_aug0")
            v_aug1 = big.tile([128, NK, 65], BF16, name="v_aug1")
            attn_pair = big.tile([128, T], BF16, name="attn_pair")
            attn1_tmp = big.tile([64, NT, CH], BF16, name="attn1_tmp")
            ident_sb = big.tile([128, 128], BF16, name="ident_sb")
            ones_nrm = big.tile([128, 128], F32R, name="ones_nrm")
            sums_sb = big.tile([128, NT, 2, CH], F32R, name="sums_sb")
            inv_sb = big.tile([64, 2, CH], F32, name="inv_sb")
            bias_sb = big.tile([128, 3], F32, name="bias_sb")
            scr_sb = big.tile([1, 2], F32, name="scr_sb")

            # warm matmuls must not depend on any DMA (the dynamic DMA queues
            # only start delivering ~8.5us in): feed them a memset tile so the
            # solid warm block starts right when the engines come up.
            warm_sb = big.tile([128, 128], BF16, name="warm_sb")
            nc.vector.memset(warm_sb, 0.5)
            nc.sync.dma_start(out=ident_sb, in_=identb)
            nc.sync.dma_start(out=bias_sb, in_=bias)
            warm_ps = ps.tile([128, CH], F32, tag="m", bufs=2, name="warm_ps")

            def warm(n):
                if "nowarm" in _DBG:
                    return
                for _ in range(n):
                    nc.tensor.matmul(
                        warm_ps[:, 0:128], warm_sb, warm_sb, start=True, stop=True
                    )

            # preload the exp activation table during the DMA ramp (warm_sb,
            # not ident_sb: no DMA dependency)
            nc.scalar.activation(
                out=scr_sb[0:1, 0:1], in_=warm_sb[0:1, 0:1], func=EXP
            )

            # x is loaded in column halves: sweep 1 (q/k chunks 0,1) and
            # v0/v1 only touch cols 0:1024, so the critical input load is
            # w + half of x. Three queues (sync/scalar HWDGE, gpsimd SWDGE).
            # First-half x comes in 512-col pieces so the sweep's half-groups
            # (3 matmuls, ~640ns) chase the DMA stream at matching grain.
            for ct in range(NCT):
                nc.sync.dma_start(
                    out=w_sb[:, ct, :], in_=wqkv[128 * ct : 128 * ct + 128, :]
                )
                xeng = nc.scalar if ct < 4 else nc.gpsimd
                for h in (0, 1):
                    xeng.dma_start(
                        out=x_sb[:, ct, 512 * h : 512 * h + 512],
                        in_=xT[128 * ct : 128 * ct + 128, 512 * h : 512 * h + 512],
                    )
            for ct in range(NCT):
                xeng = (nc.sync, nc.scalar, nc.gpsimd)[(0, 0, 0, 0, 1, 1, 2, 2)[ct]]
                xeng.dma_start(
                    out=x_sb[:, ct, 1024:2048],
                    in_=xT[128 * ct : 128 * ct + 128, 1024:2048],
                )
            nc.sync.dma_start(out=proj_sb, in_=projT)

            nc.sync.dma_start(out=ones_nrm, in_=normones)
            nc.vector.memset(v_aug0[:, :, 64:65], 1.0)
            nc.vector.memset(v_aug1[:, :, 64:65], 1.0)

            # ---- stage B: q/k projections ----------------------------------
            # Sweep 1 (q/k for chunks 0,1) runs up front, chasing the x DMAs;
            # sweep 2 is emitted in halves inside chunk 1's j-loop as PE
            # filler (each half holds only one wide s slot so the scores/exp
            # pipeline keeps the other).
            def qk_part(gs, part, cts, evac=False):
                # 1-bank m-tag groups so the scores/exp pipeline keeps both
                # wide s slots; emitted a few ct-steps per j as PE filler.
                cols = slice(128 * part, 128 * part + 128)
                for ct in cts:
                    for c, g in gs.items():
                        nc.tensor.matmul(
                            g,
                            w_sb[:, ct, cols],
                            x_sb[:, ct, CH * c : CH * c + CH],
                            start=(ct == 0),
                            stop=(ct == NCT - 1),
                        )
                if evac:
                    dest = qT_sb if part == 0 else kT_sb
                    for c, g in gs.items():
                        nc.vector.tensor_scalar_add(
                            dest[:, CH * c : CH * c + CH],
                            g,
                            bias_sb[:, part : part + 1],
                        )

            def qk_groups(tag_name):
                return {
                    c: ps.tile([128, CH], F32, tag="m", bufs=2, name=f"{tag_name}{c}")
                    for c in (2, 3)
                }

            # sweep 1: interleave q and k groups across both s slots, ct-outer
            # so the matmuls chase the x DMA stream. The warmup block is one
            # solid ~6.9us burst: HAM's free-running activity window needs a
            # fully-covered busy window, which a burst only guarantees when it
            # spans ~two windows.
            warm(N_WARM)
            grp = {}
            for part in (0, 1):
                g = ps.tile([128, 2, CH], F32, tag="s", bufs=2, name=f"qkps_{part}")
                for ci in (0, 1):
                    grp[(part, ci)] = g[:, ci, :]
            # v chunks 0 and 1 also read only x cols 0:1024, so their matmuls
            # ride along in the ct chase: 6 matmuls (~1.3us) per arriving x
            # tile keeps the PE dense (and HAM warm) through the input phase.
            v0_ps = ps.tile([128, CH], F32, tag="m", bufs=2, name="v0_ps")
            v1_ps = ps.tile([128, CH], F32, tag="m", bufs=2, name="v1_ps")
            for ct in range(NCT):
                # per arriving 512-col x piece: q, k, and v matmuls for that
                # chunk (3 matmuls ~640ns) — half-group grain matches the DMA
                # piece grain so the PE chases the stream nearly gaplessly.
                for c, v_ps in ((0, v0_ps), (1, v1_ps)):
                    for part in (0, 1):
                        nc.tensor.matmul(
                            grp[(part, c)],
                            w_sb[:, ct, 128 * part : 128 * part + 128],
                            x_sb[:, ct, CH * c : CH * c + CH],
                            start=(ct == 0),
                            stop=(ct == NCT - 1),
                        )
                    nc.tensor.matmul(
                        v_ps,
                        w_sb[:, ct, 256:384],
                        x_sb[:, ct, CH * c : CH * c + CH],
                        start=(ct == 0),
                        stop=(ct == NCT - 1),
                    )
                    if ct < NCT - 1 or c == 0:
                        warm(2)
            for (part, c), g in grp.items():
                dest = qT_sb if part == 0 else kT_sb
                nc.vector.tensor_scalar_add(
                    dest[:, CH * c : CH * c + CH], g, bias_sb[:, part : part + 1]
                )
            for c, v_ps in ((0, v0_ps), (1, v1_ps)):
                nc.vector.tensor_scalar_add(
                    vT_sb[:, CH * c : CH * c + CH], v_ps, bias_sb[:, 2:3]
                )

            # v projection + PE transposes, per chunk; chunks 0/1 up front,
            # the rest emitted as PE filler inside stage D's j-loops.
            def emit_v_chunk(c):
                v_ps = ps.tile([128, CH], F32, tag="m", bufs=2, name=f"vps_{c}")
                for ct in range(NCT):
                    nc.tensor.matmul(
                        v_ps,
                        w_sb[:, ct, 256:384],
                        x_sb[:, ct, CH * c : CH * c + CH],
                        start=(ct == 0),
                        stop=(ct == NCT - 1),
                    )
                nc.vector.tensor_scalar_add(
                    vT_sb[:, CH * c : CH * c + CH], v_ps, bias_sb[:, 2:3]
                )

            def transposes_for(c, half=None):
                # one 128x128 transpose per k-tile covers both heads:
                # out cols 0-63 = head-0 dims, 64-127 = head-1 dims.
                kts = range(4 * c, 4 * c + 4)
                if half is not None:
                    kts = kts[2 * half : 2 * half + 2]
                for kt in kts:
                    if "notr128" in _DBG:
                        for h, v_aug in ((0, v_aug0), (1, v_aug1)):
                            hrow = slice(64 * h, 64 * h + 64)
                            tr_ps = ps.tile(
                                [128, 64], BF16, tag="m", bufs=2, name=f"tr_{h}_{kt}"
                            )
                            nc.tensor.transpose(
                                tr_ps,
                                vT_sb[hrow, 128 * kt : 128 * kt + 128],
                                ident_sb[hrow, 0:64],
                            )
                            nc.vector.tensor_copy(v_aug[:, kt, 0:64], tr_ps)
                        continue
                    tr_ps = ps.tile([128, 128], BF16, tag="m", bufs=2, name=f"tr_{kt}")
                    nc.tensor.transpose(
                        tr_ps,
                        vT_sb[:, 128 * kt : 128 * kt + 128],
                        ident_sb,
                    )
                    nc.vector.tensor_copy(v_aug0[:, kt, 0:64], tr_ps[:, 0:64])
                    nc.vector.tensor_copy(v_aug1[:, kt, 0:64], tr_ps[:, 64:128])

            transposes_for(0)

            # ---- stages D+E per t-chunk ------------------------------------
            # Each chunk's norm is emitted at its own end; its projection
            # tiles are deferred and spread through the next chunk's j-loop.
            # cross-chunk projection queue: (pc, m, epoch). Tiles are pushed
            # after their chunk's norm and drained ~1 per j-step of later
            # chunks, so no chunk ends with a flush that walls off the next
            # chunk's scores on the in-order PE queue.
            proj_queue = []

            def emit_norm(pvA, pvB, pc):
                # head 1 first: its normalized rows must still hop partitions
                # (DVE lanes are partition-locked, so an SBUF->SBUF DMA moves
                # them from 0:64 to 64:128); starting the h1 chain first lets
                # head 0's recip/mul overlap the shift. The shift rides the
                # sync HW queue — the gpsimd SW queue adds a ~2us drain.
                tcol = slice(CH * pc, CH * pc + CH)
                nc.vector.tensor_copy(sums_sb[64:65, pc, 1, :], pvB[64:65, :])
                rb1_ps = ps.tile([128, CH], F32, tag="m", bufs=2, name=f"rb1_{pc}")
                nc.tensor.matmul(
                    rb1_ps[0:64, :],
                    ones_nrm[64:65, 0:64],
                    sums_sb[64:65, pc, 1, :],
                    start=True,
                    stop=True,
                )
                nc.vector.reciprocal_approx_fast(
                    out=inv_sb[:, 1, :], in_=rb1_ps[0:64, :]
                )
                nc.vector.tensor_mul(
                    attn1_tmp[:, pc, :], pvB[0:64, :], inv_sb[:, 1, :]
                )
                nc.sync.dma_start(
                    out=attn_pair[64:128, tcol], in_=attn1_tmp[:, pc, :]
                )
                nc.vector.tensor_copy(sums_sb[64:65, pc, 0, :], pvA[64:65, :])
                rb0_ps = ps.tile([128, CH], F32, tag="m", bufs=2, name=f"rb0_{pc}")
                nc.tensor.matmul(
                    rb0_ps[0:64, :],
                    ones_nrm[64:65, 0:64],
                    sums_sb[64:65, pc, 0, :],
                    start=True,
                    stop=True,
                )
                nc.vector.reciprocal_approx_fast(
                    out=inv_sb[:, 0, :], in_=rb0_ps[0:64, :]
                )
                nc.vector.tensor_mul(
                    attn_pair[0:64, tcol], pvA[0:64, :], inv_sb[:, 0, :]
                )

            def emit_proj_tile(pc, m, cast_eng=None, store_eng=None):
                tcol = slice(CH * pc, CH * pc + CH)
                pr_ps = ps.tile([128, CH], F32, tag="m", bufs=2, name=f"pr_{m}_{pc}")
                nc.tensor.matmul(
                    pr_ps,
                    proj_sb[:, 128 * m : 128 * m + 128],
                    attn_pair[:, tcol],
                    start=True,
                    stop=True,
                )
                ob = outev_pool.tile([128, CH], BF16, tag="outev", name=f"ob_{m}_{pc}")
                if cast_eng is nc.scalar and "noscalcast" not in _DBG:
                    nc.scalar.copy(ob, pr_ps)
                else:
                    nc.vector.tensor_copy(ob, pr_ps)
                (store_eng or nc.sync).dma_start(out=out[pc, m], in_=ob)

            # Chunk order (1, 2, 0, 3): the last chunk processed is the
            # longest one, so both PE (scores+pv+proj filler) and ScalarE
            # (16 exps) stay dense right up to the tail — HAM never sees an
            # idle window until the final norm+proj. The short chunk 0 sits
            # mid-kernel where chunk 2's projection tiles pack its PE slack.
            chunk_order = (1, 2, 0, 3)
            for f in _DBG:
                if f.startswith("c") and f[1:].isdigit():
                    chunk_order = (1, 2, 0, 3)[: int(f[1:])]
            for ci, c in enumerate(chunk_order):
                nj = 4 * c + 4
                pvA = ps.tile([128, CH], F32, tag="pv", bufs=2, name=f"pvA_{c}")
                pvB = ps.tile([128, CH], F32, tag="pv", bufs=2, name=f"pvB_{c}")
                pending = []

                def emit_pv(item, last):
                    pj, pw, plo = item
                    # both v_aug tiles carry a ones column at index 64, so
                    # each head's denominator lands at row 64 of its pv bank
                    # with no extra matmul.
                    nc.tensor.matmul(
                        pvA[0:65, plo:CH],
                        v_aug0[:, pj, :],
                        pw[:, 0, plo:CH],
                        start=(pj == 0),
                        stop=last,
                    )
                    nc.tensor.matmul(
                        pvB[0:65, plo:CH],
                        v_aug1[:, pj, :],
                        pw[:, 1, plo:CH],
                        start=(pj == 0),
                        stop=last,
                    )

                for j in range(nj):
                    if "nowides" in _DBG:
                        sa = ps.tile([128, CH], F32, tag="s", bufs=4, name=f"sa_{c}_{j}")
                        sb = ps.tile([128, CH], F32, tag="s", bufs=4, name=f"sb_{c}_{j}")
                        s_ps = _FakeWide((sa, sb))
                    else:
                        s_ps = ps.tile(
                            [128, 2, CH], F32, tag="s", bufs=2, name=f"s_{c}_{j}"
                        )
                    # diagonal tiles: columns < 128*diag are fully masked
                    # downstream, so don't compute their scores either
                    diag = j - 4 * c
                    slo = max(0, 128 * diag)
                    for h in (0, 1):
                        hrow = slice(64 * h, 64 * h + 64)
                        nc.tensor.matmul(
                            s_ps[:, h, slo:CH],
                            kT_sb[hrow, 128 * j : 128 * j + 128],
                            qT_sb[hrow, CH * c + slo : CH * c + CH],
                            start=True,
                            stop=True,
                        )
                    w_t = expw_pool.tile(
                        [128, 2, CH], BF16, tag="expw", name=f"w_{c}_{j}"
                    )
                    # one wide exp per j covers both heads (2 PSUM banks)
                    if "nowide" in _DBG or "nowides" in _DBG:
                        for h in (0, 1):
                            nc.scalar.activation(
                                out=w_t[:, h, slo:CH], in_=s_ps[:, h, slo:CH], func=EXP
                            )
                    else:
                        nc.scalar.activation(
                            out=w_t[:, :, slo:CH], in_=s_ps[:, :, slo:CH], func=EXP
                        )
                    if diag >= 0:
                        # keep exp(score) where t >= k: within the kept column
                        # range f' = f - 128*diag, so keep f' - p >= 0
                        for h in (0, 1):
                            nc.gpsimd.affine_select(
                                out=w_t[:, h, slo:CH],
                                in_=w_t[:, h, slo:CH],
                                pattern=[[1, CH - slo]],
                                compare_op=IS_GE,
                                fill=0.0,
                                base=0,
                                channel_multiplier=-1,
                            )
                    pending.append((j, w_t, slo))
                    if c == 0 and "nokeep" not in _DBG:
                        # chunk 0 has almost no PE filler; keepalives stop
                        # HAM from re-throttling through its 4 exp steps
                        warm(4)
                    if c == 1:
                        # sweep 2 (q/k chunks 2,3) + v1 as chunk 1's filler,
                        # a few hundred ns of PE work per j-step
                        if j == 0:
                            qk2q = qk_groups("qk2q")
                            qk_part(qk2q, 0, range(0, 4))
                        if j == 1:
                            qk_part(qk2q, 0, range(4, 8), evac=True)
                        if j == 2:
                            qk2k = qk_groups("qk2k")
                            qk_part(qk2k, 1, range(0, 4))
                        if j == 3:
                            qk_part(qk2k, 1, range(4, 8), evac=True)
                        if j == 4:
                            transposes_for(1)
                    if j == 1 and c == 2:
                        emit_v_chunk(2)
                    if j == 2 and c == 2:
                        transposes_for(2)
                    if j == 1 and c == 3:
                        emit_v_chunk(3)
                    if j == 3 and c == 3:
                        transposes_for(3)
                    while len(pending) > PIPE:
                        item, pending = pending[0], pending[1:]
                        emit_pv(item, last=False)
                    # drain the proj queue ~1 tile/step. A tile pushed at the
                    # immediately-previous boundary waits until j>=3 so its
                    # matmul never gates on the still-running norm chain
                    # (an unmet attn_pair dep stalls the in-order PE queue).
                    took = 0
                    while proj_queue and took < (2 if len(proj_queue) > nj - j else 1):
                        pc, m, epoch = proj_queue[0]
                        if epoch == ci - 1 and j < 3:
                            break
                        proj_queue.pop(0)
                        ce = nc.scalar if (c == 0 and m % 2) else None
                        emit_proj_tile(pc, m, cast_eng=ce)
                        took += 1
                while pending:
                    item, pending = pending[0], pending[1:]
                    emit_pv(item, last=(len(pending) == 0))
                # norm as early as its inputs allow (right after the final
                # stop=True PV); this chunk's proj tiles queue up behind it.
                emit_norm(pvA, pvB, c)
                proj_queue.extend((c, m, ci) for m in range(8))

            # tail: whatever projection tiles remain (the final chunk's 8);
            # casts and stores alternate Scalar/Vector + sync/scalar queues
            # (ScalarE is idle once the exps are done)
            if "notail" not in _DBG:
                if "nokeep" not in _DBG:
                    # HAM keepalive through the tail: dummy matmuls into a
                    # fresh s-slot (no tail readers of the s banks)
                    tw = ps.tile([128, 2, CH], F32, tag="s", bufs=2, name="tailwarm")
                for i, (pc, m, epoch) in enumerate(proj_queue):
                    emit_proj_tile(
                        pc,
                        m,
                        cast_eng=nc.scalar if i % 2 else None,
                        store_eng=(nc.scalar, nc.sync, nc.gpsimd)[i % 3],
                    )
                    if "nokeep" not in _DBG:
                        for _ in range(2):
                            nc.tensor.matmul(
                                tw[:, 0, 0:128],
                                warm_sb,
                                warm_sb,
                                start=True,
                                stop=True,
                            )
                proj_queue.clear()

    nc.compile()
    return nc


def _get_nc():
    if "nc" not in _CACHE:
        _CACHE["nc"] = _build()
    return _CACHE["nc"]


def _make_in_maps(x, wqkv_w, wqkv_b, proj_w):
    bf = ml_dtypes.bfloat16
    xT = np.ascontiguousarray(np.asarray(x, np.float32).T.astype(bf))
    identb = np.eye(128, dtype=bf)
    scale = np.float32(1.0 / np.sqrt(C))
    in_maps = []
    for i in range(N_CORES):
        rows = []
        biases = []
        for blk, s in ((0, scale), (1, None), (2, None)):
            sl = slice(blk * C + 128 * i, blk * C + 128 * i + 128)
            w = np.asarray(wqkv_w[sl], np.float32)
            b = np.asarray(wqkv_b[sl], np.float32)
            if s is not None:
                w = w * s
                b = b * s
            rows.append(w)
            biases.append(b)
        W = np.concatenate(rows, axis=0)  # [384, 1024]
        B = np.stack(biases, axis=1)  # [128, 3]
        pT = np.asarray(proj_w[:, 128 * i : 128 * i + 128], np.float32).T  # [128, 1024]
        normones = np.zeros((128, 128), np.float32)
        normones[0] = 1.0
        normones[64] = 1.0
        in_maps.append(
            {
                "xT": xT,
                "wqkv": np.ascontiguousarray(W.T.astype(bf)),
                "projT": np.ascontiguousarray(pT.astype(bf)),
                "identb": identb,
                "normones": normones,
                "bias": np.ascontiguousarray(B),
            }
        )
    return in_maps


def kernel(x, wqkv_w, wqkv_b, proj_w, proj_b, _trace=False, _tmpdir=None):
    from concourse.bass_utils import run_bass_kernel_spmd

    nc = _get_nc()
    in_maps = _make_in_maps(x, wqkv_w, wqkv_b, proj_w)
    res = run_bass_kernel_spmd(
        nc,
        in_maps,
        core_ids=list(range(N_CORES)),
        trace=_trace,
        tmpdir=_tmpdir,
    )
    acc = np.zeros((NT, 8, 128, CH), np.float64)
    for rmap in res.results:
        acc += rmap["out"].astype(np.float64)
    partialT = acc.transpose(1, 2, 0, 3).reshape(C, T)  # [o, t]
    full = partialT.T + np.asarray(proj_b, np.float64)[None, :]
    if _trace:
        _CACHE["last_result"] = res
    return full.astype(np.float32)

